# revision 23
# baseline (speedup 1.0000x reference)
"""Trainium2 Bass kernel for nn_GATrAutoRegressorLoss.

Strategy v4 (data-parallel over packed valid elements, 8 cores):

  - Assignment BCE numerator = sum over valid (t,hit) of softplus(x) - x*z.
    Since z selects exactly one valid t per hit and softplus(x) - x =
    softplus(-x), the host NEGATES the selected logits; the numerator is
    then a plain sum of softplus over the valid elements only.
  - The host packs ONLY the valid elements (~50% of T*N; validity is an
    index computation on hit_batch/gt_batch) into a flat stream, padded
    with -96, sharded evenly over 8 cores as (128, W) bf16 slabs.  No
    masks, no matmuls, no PE work at all.
  - softplus sum via log-domain pairwise folding, split across engines:
      ACT: u = exp(x)                               (1 pass, all cols)
      DVE: v = u/8 + 1/8                            (tensor_scalar, 4x mode)
      DVE: three halving tensor_tensor multiplies   (2x mode)
      ACT: ln(group-of-8 products) + accum_out      (W/8 cols, batched
           over chunks 0-2 and chunk 3 -> only two accumulator reads)
    Each slot contributes softplus(x_i) - 3ln2 to the ln sums; the host
    adds 3ln2 per slot.  Padding slots give v=1/8 exactly, contributing
    exactly 0 after the correction.  Group products stay in range:
    (1+e^5.7)^8/8^8 ~ e^29, all-padding groups 8^-8 ~ e^-16.6.
  - The small (T,B) losses are restructured mask-free (host folds the
    valid mask into the data: gt==pred at invalid slots, pid logits
    zeroed with a ln(5) count correction, stop logits sign-flipped by
    gt_stop) and sharded over cores by event.  Square-sums run as DVE
    scalar_tensor_tensor accums (cheap accumulator reads), the three
    k=3 plane reductions are merged into one tensor_reduce.
  - The Tile epilogue normally zeroes all ~250 reserved semaphores on
    every engine (~6us); this kernel's NEFF is compiled and executed
    once per call, so the exit-time clear is skipped.
  - Per-core partial sums are combined on the host in float64.
"""

import numpy as np

import concourse.bacc as bacc
import concourse.mybir as mybir
from concourse.tile import TileContext
from concourse.vector_clock import ScopedClock
from concourse.bass_utils import run_bass_kernel_spmd

F32 = mybir.dt.float32
BF16 = mybir.dt.bfloat16
F8 = mybir.dt.float8e4
NP_BF16 = mybir.dt.np(BF16)
NP_F8 = mybir.dt.np(F8)

T, B, N, NPFO = 32, 256, 500000, 4096
L_DIR, L_MAG, L_PID, L_CHG, L_ASN, L_STP = 1.0, 1.0, 1.0, 0.5, 1.0, 0.5

N_CORES = 8
P = 128
PAD = -96.0
LN2X3 = 3.0 * np.log(2.0)
NCH = 6            # stream chunks per core

# small-loss planes, per core (T, B/8) = (32, 32) -> (128, 8)
SW = 8
EV = B // N_CORES
_PLANES = ["pm0", "pm1", "pm2", "pp", "pch",
           "gm0", "gm1", "gm2", "gp", "gch",
           "pid0", "pid1", "pid2", "pid3", "pid4",
           "sxf", "sel"]
NPL = len(_PLANES)

_nc_cache = {}
last_result = None


class _Bacc(bacc.Bacc):
    """Pin Exp/Ln to the one ACT table containing both (plus Square), so
    the Scalar engine loads its function table exactly once."""

    def insert_act_table_loads(self):
        from concourse.hw_specs import get_activation_tables

        has_activation = any(
            isinstance(i, mybir.InstActivation)
            for b in self.main_func.blocks
            for i in b.instructions
        )
        if not has_activation:
            return
        AF = mybir.ActivationFunctionType
        tables = []
        for name, fns in get_activation_tables(self.m.arch).items():
            if name != "natural_log_exp_and_others":
                fns = set()
            tables.append((name, set(fns)))
        import bass_rust as _bass_rust

        _bass_rust.insert_act_table_loads(self, tables)


class _TC(TileContext):
    """TileContext whose epilogue skips the exit-time semaphore clearing
    loop (~250 per-semaphore instructions spread over all engines).  The
    NEFF built here is compiled and executed exactly once per kernel()
    call, so leaving the semaphores set is safe."""

    def _drain_and_barrier(self, tick_clock, wait_clock):
        drain_inst = self.nc.sync.drain()
        wait_clock.add_sem_waits(
            drain_inst.ins, ScopedClock({None: tick_clock.global_clock})
        )
        self.nc.all_engine_barrier()
        assert self.sems is not None
        popped = self.nc._tile_sem_poison_stack.pop()
        assert popped is self._sem_poison
        self.nc.all_engine_barrier()


def _chunks(W):
    # six ascending chunks alternating between the two hardware DGE
    # queues (sync: 0/2/4, scalar: 1/3/5) so transfers run in parallel
    # and each chunk lands just before the ACT stream needs it.
    base = [512, 1008, 1504, 1792, 1792, 1216]
    s = W / float(sum(base))
    ws = [max(16, int(b * s) & ~15) for b in base[:-1]]
    ws.append(W - sum(ws))
    assert all(x % 16 == 0 and x > 0 for x in ws) and sum(ws) == W
    off = np.cumsum([0] + ws[:-1]).tolist()
    return list(zip(off, ws))


def _gen(W):
    """Build the SPMD kernel for per-core slab (128, W) bf16.
    W must be a multiple of 16."""
    CH = _chunks(W)
    wmax = max(w for _, w in CH)
    fr_a = sum(w for _, w in CH[:5]) // 8
    fr_b = CH[5][1] // 8

    nc = _Bacc(None, target_bir_lowering=False, debug=True)
    xb = nc.dram_tensor("xb", [P, W], BF16, kind="ExternalInput")
    sm = nc.dram_tensor("sm", [P, NPL * SW], F32, kind="ExternalInput")
    pa = nc.dram_tensor("pa", [P, 4], F32, kind="ExternalOutput")
    pd = nc.dram_tensor("pd", [P, 6], F32, kind="ExternalOutput")

    AF = mybir.ActivationFunctionType
    OP = mybir.AluOpType

    with _TC(nc) as tc:
        with (
            tc.tile_pool(name="io", bufs=1) as io,
            tc.tile_pool(name="wk", bufs=1) as wk,
        ):
            xt = [io.tile([P, w], BF16, name=f"xt{i}", tag=f"xt{i}")
                  for i, (_, w) in enumerate(CH)]
            smt = io.tile([P, NPL * SW], F32)
            ut = [wk.tile([P, wmax], BF16, name=f"ut{i}", tag=f"ut{i}")
                  for i in range(2)]
            fs = wk.tile([P, wmax // 2 + wmax // 4], BF16)
            fra = wk.tile([P, fr_a], BF16)
            frb = wk.tile([P, fr_b], BF16)
            accA = wk.tile([P, 4], F32)
            accD = wk.tile([P, 6], F32)
            # scratch written only by ACT / only by DVE (column layout)
            # scrA: sq 0:48 | epu 48:96 | lnb 96:112 | srb 112:128 |
            #       lset 128:136 | spvt 136:144
            scrA = wk.tile([P, 144], F32)
            # scrD: red 0:24 (ssb 0:16, dott 16:24) | dmul 24:48 |
            #       sered 48:56 | dsub 56:72 | selo 72:80 | c1 80:88 |
            #       c2 88:96
            scrD = wk.tile([P, 96], F32)

            def pl(name, k=1):
                i = _PLANES.index(name)
                return smt[:, i * SW: (i + k) * SW]

            def folds(ci):
                """DVE part of chunk ci: scale+shift then 3 halving mults,
                final product into fra/frb at the chunk's offset."""
                u = ut[ci % 2]
                w = CH[ci][1]
                nc.vector.tensor_scalar(
                    out=u[:, :w], in0=u[:, :w], scalar1=0.125, scalar2=0.125,
                    op0=OP.mult, op1=OP.add)
                h = w // 2
                nc.vector.tensor_mul(fs[:, :h], u[:, :h], u[:, h:2 * h])
                q = h // 2
                nc.vector.tensor_mul(fs[:, h:h + q], fs[:, :q], fs[:, q:2 * q])
                g = q // 2
                if ci < 5:
                    off = CH[ci][0] // 8
                    dst = fra[:, off:off + g]
                else:
                    dst = frb[:]
                nc.vector.tensor_mul(dst, fs[:, h:h + g], fs[:, h + g:h + q])

            # ---- DMA triggers spread across four queues
            def xdma(eng, i):
                eng.dma_start(out=xt[i][:],
                              in_=xb[:, CH[i][0]: CH[i][0] + CH[i][1]])

            xdma(nc.sync, 0)
            xdma(nc.scalar, 1)
            nc.gpsimd.dma_start(out=smt[:], in_=sm[:])
            xdma(nc.sync, 2)
            xdma(nc.sync, 4)

            # ---- chunk 0 / 1 (later scalar-queue configs slot into ACT's
            # arrival-wait gaps so they don't delay exp0)
            nc.scalar.activation(out=ut[0][:, :CH[0][1]], in_=xt[0][:], func=AF.Exp)
            xdma(nc.scalar, 3)
            folds(0)
            nc.scalar.activation(out=ut[1][:, :CH[1][1]], in_=xt[1][:], func=AF.Exp)
            xdma(nc.scalar, 5)
            folds(1)

            # ---- ACT small front: squares of pm|gm, exp of pid+sxf
            sq_in = smt[:, 0:80].rearrange("p (g c) -> p g c", g=2)[:, :, 0:24]
            nc.scalar.activation(
                out=scrA[:, 0:48].rearrange("p (g c) -> p g c", g=2),
                in_=sq_in, func=AF.Square)
            nc.scalar.activation(out=scrA[:, 48:96], in_=pl("pid0", 6),
                                 func=AF.Exp)

            # ---- DVE small front
            nc.vector.tensor_mul(scrD[:, 24:48], pl("pm0", 3), pl("gm0", 3))
            # merged reduce over [pm^2 | gm^2] -> [ssb_p | ssb_g], then dot
            nc.vector.tensor_reduce(
                out=scrD[:, 0:16].rearrange("p (g j) -> p g j", g=2),
                in_=scrA[:, 0:48].rearrange("p (g k j) -> p g j k", g=2, k=3),
                axis=mybir.AxisListType.X, op=OP.add)
            nc.vector.tensor_reduce(
                out=scrD[:, 16:24],
                in_=scrD[:, 24:48].rearrange("p (k j) -> p j k", k=3),
                axis=mybir.AxisListType.X, op=OP.add)
            nc.vector.tensor_reduce(
                out=scrD[:, 48:56],
                in_=scrA[:, 48:88].rearrange("p (k j) -> p j k", k=5),
                axis=mybir.AxisListType.X, op=OP.add)
            nc.vector.tensor_scalar(
                out=scrD[:, 0:16], in0=scrD[:, 0:16], scalar1=1e-16,
                scalar2=None, op0=OP.max)
            nc.vector.tensor_sub(scrD[:, 56:72], smt[:, 24:40], smt[:, 64:80])
            nc.vector.tensor_scalar(
                out=scrD[:, 72:80], in0=pl("sel"), scalar1=1.0, scalar2=0.0,
                op0=OP.mult, op1=OP.add, accum_out=accD[:, 0:1])
            nc.vector.scalar_tensor_tensor(
                out=scrD[:, 80:88], in0=scrD[:, 56:64], scalar=1.0,
                in1=scrD[:, 56:64], op0=OP.mult, op1=OP.mult,
                accum_out=accD[:, 2:3])
            nc.vector.scalar_tensor_tensor(
                out=scrD[:, 88:96], in0=scrD[:, 64:72], scalar=1.0,
                in1=scrD[:, 64:72], op0=OP.mult, op1=OP.mult,
                accum_out=accD[:, 3:4])

            # ---- chunk 2 + ACT small back
            nc.scalar.activation(out=ut[0][:, :CH[2][1]], in_=xt[2][:], func=AF.Exp)
            folds(2)
            nc.scalar.activation(out=scrA[:, 136:144], in_=scrA[:, 88:96],
                                 func=AF.Ln, bias=1.0, accum_out=accA[:, 2:3])
            nc.scalar.activation(out=scrA[:, 96:112], in_=scrD[:, 0:16],
                                 func=AF.Ln)
            nc.scalar.activation(out=scrA[:, 112:128], in_=scrA[:, 96:112],
                                 func=AF.Exp, scale=-0.5)
            nc.scalar.activation(out=scrA[:, 128:136], in_=scrD[:, 48:56],
                                 func=AF.Ln, accum_out=accA[:, 3:4])

            # ---- chunks 3 / 4
            nc.scalar.activation(out=ut[1][:, :CH[3][1]], in_=xt[3][:], func=AF.Exp)
            folds(3)
            nc.scalar.activation(out=ut[0][:, :CH[4][1]], in_=xt[4][:], func=AF.Exp)
            folds(4)
            nc.vector.tensor_mul(scrD[:, 80:88], scrD[:, 16:24],
                                 scrA[:, 112:120])
            nc.vector.scalar_tensor_tensor(
                out=scrD[:, 88:96], in0=scrD[:, 80:88], scalar=-1.0,
                in1=scrA[:, 120:128], op0=OP.mult, op1=OP.mult,
                accum_out=accD[:, 1:2])
            nc.gpsimd.dma_start(out=pd[:], in_=accD[:])

            # ---- chunk 5 + deferred stream lns
            nc.scalar.activation(out=ut[1][:, :CH[5][1]], in_=xt[5][:], func=AF.Exp)
            nc.scalar.activation(out=fra[:], in_=fra[:], func=AF.Ln,
                                 accum_out=accA[:, 0:1])
            folds(5)
            nc.scalar.activation(out=frb[:], in_=frb[:], func=AF.Ln,
                                 accum_out=accA[:, 1:2])
            nc.sync.dma_start(out=pa[:], in_=accA[:])
    nc.finalize()
    return nc


def _get_nc(W):
    if W not in _nc_cache:
        _nc_cache[W] = _gen(W)
    return _nc_cache[W]


def _cumcount(gb):
    n = gb.shape[0]
    order = np.argsort(gb, kind="stable")
    sb = gb[order]
    first = np.searchsorted(sb, sb, side="left")
    cum = np.arange(n) - first
    out = np.zeros(n, dtype=np.int64)
    out[order] = cum
    return out


def kernel(**inputs):
    pfo_momentum = np.asarray(inputs["pfo_momentum"], np.float32)
    pfo_p_mod = np.asarray(inputs["pfo_p_mod"], np.float32)
    pfo_pid = np.asarray(inputs["pfo_pid"], np.float32)
    pfo_charge = np.asarray(inputs["pfo_charge"], np.float32)
    al = np.asarray(inputs["assignments_logits"], np.float32).reshape(T, N)
    stop_logits = np.asarray(inputs["stop_logits"], np.float32)
    gt_momentum = np.asarray(inputs["gt_momentum"], np.float32)
    gt_p_mod = np.asarray(inputs["gt_p_mod"], np.float32)
    gt_pid = np.asarray(inputs["gt_pid"], np.float32)
    gt_charge = np.asarray(inputs["gt_charge"], np.float32)
    gt_batch = np.asarray(inputs["gt_batch"]).astype(np.int64)
    hit_to_pfo = np.asarray(inputs["hit_to_pfo"]).astype(np.int64)
    hit_batch = np.asarray(inputs["hit_batch"]).astype(np.int64)

    # ---- assignment stream: host packs valid elements, negating selected
    ppe = np.bincount(gt_batch, minlength=B)[:B]
    c = np.minimum(ppe[hit_batch], T)                              # (N,)
    w = hit_to_pfo < c
    den = max(float(c.sum()), 1.0)

    als = al.copy()
    idx = np.nonzero(w)[0]
    als[hit_to_pfo[idx], idx] = -als[hit_to_pfo[idx], idx]
    mask = np.arange(T)[:, None] < c[None, :]                      # (T, N)
    vals = als[mask]                                               # (K,) t-major
    K = vals.size

    gran = N_CORES * P * 16
    total = max(-(-K // gran), 16) * gran
    W = total // (N_CORES * P)                                     # cols per core
    buf = np.full(total, PAD, np.float32)
    buf[:K] = vals
    slabs = buf.reshape(N_CORES, P, W).astype(NP_BF16)

    # ---- small (T,B) losses: mask-free planes
    step_idx = _cumcount(gt_batch)
    keep = step_idx < T
    si, gb = step_idx[keep], gt_batch[keep]

    def scat(v):
        out = np.zeros((T, B) + v.shape[1:], np.float32)
        out[si, gb] = v[keep]
        return out

    gt_mom_tb = scat(gt_momentum)
    gt_pmod_tb = scat(gt_p_mod)
    gt_pid_tb = scat(gt_pid)
    gt_chg_tb = scat(gt_charge)

    steps = np.arange(T)[:, None]
    valid = (steps < ppe[None, :])                                 # (T,B) bool
    vcnt = max(float(valid.sum()), 1.0)
    ninv = T * B - float(valid.sum())
    gt_stop = steps >= ppe[None, :]
    gt_cls = np.argmax(gt_pid_tb, axis=-1)
    sel = np.take_along_axis(pfo_pid, gt_cls[..., None], axis=-1)[..., 0]
    sel = np.where(valid, sel, 0.0).astype(np.float32)
    pidz = np.where(valid[..., None], pfo_pid, 0.0).astype(np.float32)
    gp2 = np.where(valid, gt_pmod_tb[..., 0], pfo_p_mod[..., 0]).astype(np.float32)
    gch2 = np.where(valid, gt_chg_tb[..., 0], pfo_charge[..., 0]).astype(np.float32)
    sxf = np.where(gt_stop, -stop_logits[..., 0], stop_logits[..., 0]).astype(np.float32)

    planes = {
        "pm0": pfo_momentum[..., 0], "pm1": pfo_momentum[..., 1],
        "pm2": pfo_momentum[..., 2],
        "pp": pfo_p_mod[..., 0], "pch": pfo_charge[..., 0],
        "gm0": gt_mom_tb[..., 0], "gm1": gt_mom_tb[..., 1],
        "gm2": gt_mom_tb[..., 2],
        "gp": gp2, "gch": gch2,
        **{f"pid{k}": pidz[..., k] for k in range(5)},
        "sxf": sxf, "sel": sel,
    }

    in_maps = []
    for ci in range(N_CORES):
        ev = slice(ci * EV, (ci + 1) * EV)
        smc = np.concatenate(
            [np.ascontiguousarray(planes[n][:, ev]).reshape(P, SW)
             for n in _PLANES], axis=1).astype(np.float32)
        in_maps.append({"xb": np.ascontiguousarray(slabs[ci]), "sm": smc})

    nc = _get_nc(W)
    res = run_bass_kernel_spmd(nc, in_maps, core_ids=list(range(N_CORES)))
    global last_result
    last_result = res

    # ---- host combine (float64)
    A_sum = 0.0
    stop_sum = lse_sum = sel_sum = mag_sum = chg_sum = cosn_sum = 0.0
    for ci in range(N_CORES):
        pa = res.results[ci]["pa"].astype(np.float64)
        pd = res.results[ci]["pd"].astype(np.float64)
        A_sum += pa[:, 0:2].sum()
        stop_sum += pa[:, 2].sum()
        lse_sum += pa[:, 3].sum()
        sel_sum += pd[:, 0].sum()
        cosn_sum += pd[:, 1].sum()
        mag_sum += pd[:, 2].sum()
        chg_sum += pd[:, 3].sum()

    A_sum += LN2X3 * total
    loss_assign = A_sum / den
    loss_stop = stop_sum / (T * B)
    loss_pid = (lse_sum - sel_sum - ninv * np.log(5.0)) / vcnt
    loss_dir = (vcnt + cosn_sum) / vcnt
    loss_mag = mag_sum / vcnt
    loss_chg = chg_sum / vcnt

    total_loss = (L_DIR * loss_dir + L_MAG * loss_mag + L_PID * loss_pid
                  + L_CHG * loss_chg + L_ASN * loss_assign + L_STP * loss_stop)
    f = np.float32
    return (f(total_loss), f(loss_dir), f(loss_mag), f(loss_pid), f(loss_chg),
            f(loss_assign), f(loss_stop))


# revision 24
# speedup vs baseline: 1.0733x; 1.0733x over previous
"""Trainium2 Bass kernel for nn_GATrAutoRegressorLoss.

Strategy v4 (data-parallel over packed valid elements, 8 cores):

  - Assignment BCE numerator = sum over valid (t,hit) of softplus(x) - x*z.
    Since z selects exactly one valid t per hit and softplus(x) - x =
    softplus(-x), the host NEGATES the selected logits; the numerator is
    then a plain sum of softplus over the valid elements only.
  - The host packs ONLY the valid elements (~50% of T*N; validity is an
    index computation on hit_batch/gt_batch) into a flat stream, padded
    with -96, sharded evenly over 8 cores as (128, W) bf16 slabs.  No
    masks, no matmuls, no PE work at all.
  - softplus sum via log-domain pairwise folding, split across engines:
      ACT: u = exp(x)                               (1 pass, all cols)
      DVE: v = u/8 + 1/8                            (tensor_scalar, 4x mode)
      DVE: three halving tensor_tensor multiplies   (2x mode)
      ACT: ln(group-of-8 products) + accum_out      (W/8 cols, batched
           over chunks 0-2 and chunk 3 -> only two accumulator reads)
    Each slot contributes softplus(x_i) - 3ln2 to the ln sums; the host
    adds 3ln2 per slot.  Padding slots give v=1/8 exactly, contributing
    exactly 0 after the correction.  Group products stay in range:
    (1+e^5.7)^8/8^8 ~ e^29, all-padding groups 8^-8 ~ e^-16.6.
  - The small (T,B) losses are restructured mask-free (host folds the
    valid mask into the data: gt==pred at invalid slots, pid logits
    zeroed with a ln(5) count correction, stop logits sign-flipped by
    gt_stop) and sharded over cores by event.  Square-sums run as DVE
    scalar_tensor_tensor accums (cheap accumulator reads), the three
    k=3 plane reductions are merged into one tensor_reduce.
  - The Tile epilogue normally zeroes all ~250 reserved semaphores on
    every engine (~6us); this kernel's NEFF is compiled and executed
    once per call, so the exit-time clear is skipped.
  - Per-core partial sums are combined on the host in float64.
"""

import numpy as np

import concourse.bacc as bacc
import concourse.mybir as mybir
from concourse.tile import TileContext
from concourse.vector_clock import ScopedClock
from concourse.bass_utils import run_bass_kernel_spmd

F32 = mybir.dt.float32
BF16 = mybir.dt.bfloat16
F8 = mybir.dt.float8e4
NP_BF16 = mybir.dt.np(BF16)
NP_F8 = mybir.dt.np(F8)

T, B, N, NPFO = 32, 256, 500000, 4096
L_DIR, L_MAG, L_PID, L_CHG, L_ASN, L_STP = 1.0, 1.0, 1.0, 0.5, 1.0, 0.5

N_CORES = 8
P = 128
PAD = -96.0
LN2X3 = 3.0 * np.log(2.0)
NCH = 4            # stream chunks per core

# small-loss planes, per core (T, B/8) = (32, 32) -> (128, 8)
SW = 8
EV = B // N_CORES
_PLANES = ["pm0", "pm1", "pm2", "pp", "pch",
           "gm0", "gm1", "gm2", "gp", "gch",
           "pid0", "pid1", "pid2", "pid3", "pid4",
           "sxf", "sel"]
NPL = len(_PLANES)

_nc_cache = {}
last_result = None


class _Bacc(bacc.Bacc):
    """Pin Exp/Ln to the one ACT table containing both (plus Square), so
    the Scalar engine loads its function table exactly once."""

    def insert_act_table_loads(self):
        from concourse.hw_specs import get_activation_tables

        has_activation = any(
            isinstance(i, mybir.InstActivation)
            for b in self.main_func.blocks
            for i in b.instructions
        )
        if not has_activation:
            return
        AF = mybir.ActivationFunctionType
        tables = []
        for name, fns in get_activation_tables(self.m.arch).items():
            if name != "natural_log_exp_and_others":
                fns = set()
            tables.append((name, set(fns)))
        import bass_rust as _bass_rust

        _bass_rust.insert_act_table_loads(self, tables)


class _TC(TileContext):
    """TileContext whose epilogue skips the exit-time semaphore clearing
    loop (~250 per-semaphore instructions spread over all engines).  The
    NEFF built here is compiled and executed exactly once per kernel()
    call, so leaving the semaphores set is safe."""

    def _drain_and_barrier(self, tick_clock, wait_clock):
        drain_inst = self.nc.sync.drain()
        wait_clock.add_sem_waits(
            drain_inst.ins, ScopedClock({None: tick_clock.global_clock})
        )
        self.nc.all_engine_barrier()
        assert self.sems is not None
        popped = self.nc._tile_sem_poison_stack.pop()
        assert popped is self._sem_poison
        self.nc.all_engine_barrier()


def _chunks(W):
    # four chunks on one hardware DGE queue: a small primer for pipeline
    # spin-up, two big chunks (wide lines give the queue its best rate),
    # and a moderate tail for the drain.
    w0 = min(512, max(16, (W // 8) & ~15))
    w3 = max(16, int(W * 0.17) & ~15)
    w1 = ((W - w0 - w3) // 2) & ~15
    w2 = W - w0 - w1 - w3
    ws = [w0, w1, w2, w3]
    assert all(x % 16 == 0 and x > 0 for x in ws) and sum(ws) == W
    off = np.cumsum([0] + ws[:-1]).tolist()
    return list(zip(off, ws))


def _gen(W):
    """Build the SPMD kernel for per-core slab (128, W) bf16.
    W must be a multiple of 16."""
    CH = _chunks(W)
    wmax = max(w for _, w in CH)
    fr_a = sum(w for _, w in CH[:3]) // 8
    fr_b = CH[3][1] // 8

    nc = _Bacc(None, target_bir_lowering=False, debug=True)
    xb = nc.dram_tensor("xb", [P, W], BF16, kind="ExternalInput")
    sm = nc.dram_tensor("sm", [P, NPL * SW], F32, kind="ExternalInput")
    pa = nc.dram_tensor("pa", [P, 4], F32, kind="ExternalOutput")
    pd = nc.dram_tensor("pd", [P, 6], F32, kind="ExternalOutput")

    AF = mybir.ActivationFunctionType
    OP = mybir.AluOpType

    with _TC(nc) as tc:
        with (
            tc.tile_pool(name="io", bufs=1) as io,
            tc.tile_pool(name="wk", bufs=1) as wk,
        ):
            xt = [io.tile([P, w], BF16, name=f"xt{i}", tag=f"xt{i}")
                  for i, (_, w) in enumerate(CH)]
            smt = io.tile([P, NPL * SW], F32)
            ut = [wk.tile([P, wmax], BF16, name=f"ut{i}", tag=f"ut{i}")
                  for i in range(2)]
            fs = wk.tile([P, wmax // 2 + wmax // 4], BF16)
            fra = wk.tile([P, fr_a], BF16)
            frb = wk.tile([P, fr_b], BF16)
            accA = wk.tile([P, 4], F32)
            accD = wk.tile([P, 6], F32)
            # scratch written only by ACT / only by DVE (column layout)
            # scrA: sq 0:48 | epu 48:96 | lnb 96:112 | srb 112:128 |
            #       lset 128:136 | spvt 136:144
            scrA = wk.tile([P, 144], F32)
            # scrD: red 0:24 (ssb 0:16, dott 16:24) | dmul 24:48 |
            #       sered 48:56 | dsub 56:72 | selo 72:80 | c1 80:88 |
            #       c2 88:96
            scrD = wk.tile([P, 96], F32)

            def pl(name, k=1):
                i = _PLANES.index(name)
                return smt[:, i * SW: (i + k) * SW]

            def folds(ci):
                """DVE part of chunk ci: scale+shift then 3 halving mults,
                final product into fra/frb at the chunk's offset."""
                u = ut[ci % 2]
                w = CH[ci][1]
                nc.vector.tensor_scalar(
                    out=u[:, :w], in0=u[:, :w], scalar1=0.125, scalar2=0.125,
                    op0=OP.mult, op1=OP.add)
                h = w // 2
                nc.vector.tensor_mul(fs[:, :h], u[:, :h], u[:, h:2 * h])
                q = h // 2
                nc.vector.tensor_mul(fs[:, h:h + q], fs[:, :q], fs[:, q:2 * q])
                g = q // 2
                if ci < 3:
                    off = CH[ci][0] // 8
                    dst = fra[:, off:off + g]
                else:
                    dst = frb[:]
                nc.vector.tensor_mul(dst, fs[:, h:h + g], fs[:, h + g:h + q])

            # ---- DMA triggers spread across four queues
            def xdma(eng, i):
                eng.dma_start(out=xt[i][:],
                              in_=xb[:, CH[i][0]: CH[i][0] + CH[i][1]])

            for i in range(NCH):
                xdma(nc.sync, i)
            nc.scalar.dma_start(out=smt[:], in_=sm[:])

            # ---- chunk 0 / 1
            nc.scalar.activation(out=ut[0][:, :CH[0][1]], in_=xt[0][:], func=AF.Exp)
            folds(0)
            nc.scalar.activation(out=ut[1][:, :CH[1][1]], in_=xt[1][:], func=AF.Exp)
            folds(1)

            # ---- ACT small front: squares of pm|gm, exp of pid+sxf
            sq_in = smt[:, 0:80].rearrange("p (g c) -> p g c", g=2)[:, :, 0:24]
            nc.scalar.activation(
                out=scrA[:, 0:48].rearrange("p (g c) -> p g c", g=2),
                in_=sq_in, func=AF.Square)
            nc.scalar.activation(out=scrA[:, 48:96], in_=pl("pid0", 6),
                                 func=AF.Exp)

            # ---- DVE small front
            nc.vector.tensor_mul(scrD[:, 24:48], pl("pm0", 3), pl("gm0", 3))
            # merged reduce over [pm^2 | gm^2] -> [ssb_p | ssb_g], then dot
            nc.vector.tensor_reduce(
                out=scrD[:, 0:16].rearrange("p (g j) -> p g j", g=2),
                in_=scrA[:, 0:48].rearrange("p (g k j) -> p g j k", g=2, k=3),
                axis=mybir.AxisListType.X, op=OP.add)
            nc.vector.tensor_reduce(
                out=scrD[:, 16:24],
                in_=scrD[:, 24:48].rearrange("p (k j) -> p j k", k=3),
                axis=mybir.AxisListType.X, op=OP.add)
            nc.vector.tensor_reduce(
                out=scrD[:, 48:56],
                in_=scrA[:, 48:88].rearrange("p (k j) -> p j k", k=5),
                axis=mybir.AxisListType.X, op=OP.add)
            nc.vector.tensor_scalar(
                out=scrD[:, 0:16], in0=scrD[:, 0:16], scalar1=1e-16,
                scalar2=None, op0=OP.max)
            nc.vector.tensor_sub(scrD[:, 56:72], smt[:, 24:40], smt[:, 64:80])
            nc.vector.tensor_scalar(
                out=scrD[:, 72:80], in0=pl("sel"), scalar1=1.0, scalar2=0.0,
                op0=OP.mult, op1=OP.add, accum_out=accD[:, 0:1])
            nc.vector.scalar_tensor_tensor(
                out=scrD[:, 80:88], in0=scrD[:, 56:64], scalar=1.0,
                in1=scrD[:, 56:64], op0=OP.mult, op1=OP.mult,
                accum_out=accD[:, 2:3])
            nc.vector.scalar_tensor_tensor(
                out=scrD[:, 88:96], in0=scrD[:, 64:72], scalar=1.0,
                in1=scrD[:, 64:72], op0=OP.mult, op1=OP.mult,
                accum_out=accD[:, 3:4])

            # ---- chunk 2 + ACT small back
            nc.scalar.activation(out=ut[0][:, :CH[2][1]], in_=xt[2][:], func=AF.Exp)
            folds(2)
            nc.scalar.activation(out=scrA[:, 136:144], in_=scrA[:, 88:96],
                                 func=AF.Ln, bias=1.0, accum_out=accA[:, 2:3])
            nc.scalar.activation(out=scrA[:, 96:112], in_=scrD[:, 0:16],
                                 func=AF.Ln)
            nc.scalar.activation(out=scrA[:, 112:128], in_=scrA[:, 96:112],
                                 func=AF.Exp, scale=-0.5)
            nc.scalar.activation(out=scrA[:, 128:136], in_=scrD[:, 48:56],
                                 func=AF.Ln, accum_out=accA[:, 3:4])

            # ---- chunk 3
            nc.scalar.activation(out=ut[1][:, :CH[3][1]], in_=xt[3][:], func=AF.Exp)

            # ---- deferred stream lns
            nc.vector.tensor_mul(scrD[:, 80:88], scrD[:, 16:24],
                                 scrA[:, 112:120])
            nc.vector.scalar_tensor_tensor(
                out=scrD[:, 88:96], in0=scrD[:, 80:88], scalar=-1.0,
                in1=scrA[:, 120:128], op0=OP.mult, op1=OP.mult,
                accum_out=accD[:, 1:2])
            nc.gpsimd.dma_start(out=pd[:], in_=accD[:])
            nc.scalar.activation(out=fra[:], in_=fra[:], func=AF.Ln,
                                 accum_out=accA[:, 0:1])
            folds(3)
            nc.scalar.activation(out=frb[:], in_=frb[:], func=AF.Ln,
                                 accum_out=accA[:, 1:2])
            nc.sync.dma_start(out=pa[:], in_=accA[:])
    nc.finalize()
    return nc


def _get_nc(W):
    if W not in _nc_cache:
        _nc_cache[W] = _gen(W)
    return _nc_cache[W]


def _cumcount(gb):
    n = gb.shape[0]
    order = np.argsort(gb, kind="stable")
    sb = gb[order]
    first = np.searchsorted(sb, sb, side="left")
    cum = np.arange(n) - first
    out = np.zeros(n, dtype=np.int64)
    out[order] = cum
    return out


def kernel(**inputs):
    pfo_momentum = np.asarray(inputs["pfo_momentum"], np.float32)
    pfo_p_mod = np.asarray(inputs["pfo_p_mod"], np.float32)
    pfo_pid = np.asarray(inputs["pfo_pid"], np.float32)
    pfo_charge = np.asarray(inputs["pfo_charge"], np.float32)
    al = np.asarray(inputs["assignments_logits"], np.float32).reshape(T, N)
    stop_logits = np.asarray(inputs["stop_logits"], np.float32)
    gt_momentum = np.asarray(inputs["gt_momentum"], np.float32)
    gt_p_mod = np.asarray(inputs["gt_p_mod"], np.float32)
    gt_pid = np.asarray(inputs["gt_pid"], np.float32)
    gt_charge = np.asarray(inputs["gt_charge"], np.float32)
    gt_batch = np.asarray(inputs["gt_batch"]).astype(np.int64)
    hit_to_pfo = np.asarray(inputs["hit_to_pfo"]).astype(np.int64)
    hit_batch = np.asarray(inputs["hit_batch"]).astype(np.int64)

    # ---- assignment stream: host packs valid elements, negating selected
    ppe = np.bincount(gt_batch, minlength=B)[:B]
    c = np.minimum(ppe[hit_batch], T)                              # (N,)
    w = hit_to_pfo < c
    den = max(float(c.sum()), 1.0)

    als = al.copy()
    idx = np.nonzero(w)[0]
    als[hit_to_pfo[idx], idx] = -als[hit_to_pfo[idx], idx]
    mask = np.arange(T)[:, None] < c[None, :]                      # (T, N)
    vals = als[mask]                                               # (K,) t-major
    K = vals.size

    gran = N_CORES * P * 16
    total = max(-(-K // gran), 16) * gran
    W = total // (N_CORES * P)                                     # cols per core
    buf = np.full(total, PAD, np.float32)
    buf[:K] = vals
    slabs = buf.reshape(N_CORES, P, W).astype(NP_BF16)

    # ---- small (T,B) losses: mask-free planes
    step_idx = _cumcount(gt_batch)
    keep = step_idx < T
    si, gb = step_idx[keep], gt_batch[keep]

    def scat(v):
        out = np.zeros((T, B) + v.shape[1:], np.float32)
        out[si, gb] = v[keep]
        return out

    gt_mom_tb = scat(gt_momentum)
    gt_pmod_tb = scat(gt_p_mod)
    gt_pid_tb = scat(gt_pid)
    gt_chg_tb = scat(gt_charge)

    steps = np.arange(T)[:, None]
    valid = (steps < ppe[None, :])                                 # (T,B) bool
    vcnt = max(float(valid.sum()), 1.0)
    ninv = T * B - float(valid.sum())
    gt_stop = steps >= ppe[None, :]
    gt_cls = np.argmax(gt_pid_tb, axis=-1)
    sel = np.take_along_axis(pfo_pid, gt_cls[..., None], axis=-1)[..., 0]
    sel = np.where(valid, sel, 0.0).astype(np.float32)
    pidz = np.where(valid[..., None], pfo_pid, 0.0).astype(np.float32)
    gp2 = np.where(valid, gt_pmod_tb[..., 0], pfo_p_mod[..., 0]).astype(np.float32)
    gch2 = np.where(valid, gt_chg_tb[..., 0], pfo_charge[..., 0]).astype(np.float32)
    sxf = np.where(gt_stop, -stop_logits[..., 0], stop_logits[..., 0]).astype(np.float32)

    planes = {
        "pm0": pfo_momentum[..., 0], "pm1": pfo_momentum[..., 1],
        "pm2": pfo_momentum[..., 2],
        "pp": pfo_p_mod[..., 0], "pch": pfo_charge[..., 0],
        "gm0": gt_mom_tb[..., 0], "gm1": gt_mom_tb[..., 1],
        "gm2": gt_mom_tb[..., 2],
        "gp": gp2, "gch": gch2,
        **{f"pid{k}": pidz[..., k] for k in range(5)},
        "sxf": sxf, "sel": sel,
    }

    in_maps = []
    for ci in range(N_CORES):
        ev = slice(ci * EV, (ci + 1) * EV)
        smc = np.concatenate(
            [np.ascontiguousarray(planes[n][:, ev]).reshape(P, SW)
             for n in _PLANES], axis=1).astype(np.float32)
        in_maps.append({"xb": np.ascontiguousarray(slabs[ci]), "sm": smc})

    nc = _get_nc(W)
    res = run_bass_kernel_spmd(nc, in_maps, core_ids=list(range(N_CORES)))
    global last_result
    last_result = res

    # ---- host combine (float64)
    A_sum = 0.0
    stop_sum = lse_sum = sel_sum = mag_sum = chg_sum = cosn_sum = 0.0
    for ci in range(N_CORES):
        pa = res.results[ci]["pa"].astype(np.float64)
        pd = res.results[ci]["pd"].astype(np.float64)
        A_sum += pa[:, 0:2].sum()
        stop_sum += pa[:, 2].sum()
        lse_sum += pa[:, 3].sum()
        sel_sum += pd[:, 0].sum()
        cosn_sum += pd[:, 1].sum()
        mag_sum += pd[:, 2].sum()
        chg_sum += pd[:, 3].sum()

    A_sum += LN2X3 * total
    loss_assign = A_sum / den
    loss_stop = stop_sum / (T * B)
    loss_pid = (lse_sum - sel_sum - ninv * np.log(5.0)) / vcnt
    loss_dir = (vcnt + cosn_sum) / vcnt
    loss_mag = mag_sum / vcnt
    loss_chg = chg_sum / vcnt

    total_loss = (L_DIR * loss_dir + L_MAG * loss_mag + L_PID * loss_pid
                  + L_CHG * loss_chg + L_ASN * loss_assign + L_STP * loss_stop)
    f = np.float32
    return (f(total_loss), f(loss_dir), f(loss_mag), f(loss_pid), f(loss_chg),
            f(loss_assign), f(loss_stop))
